# revision 23
# baseline (speedup 1.0000x reference)
"""Trainium2 Bass kernel for nn_MoELayerStacks (moe_routing).

Full inputs in, full output out. Data-parallel over batch across 8 cores.

Math (per batch row b):
  gate = [x[:32], x[1536:1568]] @ router_w.T + router_b           # [8]
  idx  = argmax(gate)
  l1c  = x @ l1_w[e].T + l1_b[e]   for all e                      # [8, 16]
  l1x  = clip([square(l1c[:, :15])*255/256, l1c[:, :15]], 0, 1)   # [8, 30]
  l2x  = clip(l1x @ l2_w[e].T + l2_b[e], 0, 1)                    # [8, 32]
  out  = (l2x @ out_w[e].T + out_b[e] + l1c[:, 15])[idx]          # [1]

HBM-bound on reading x (12.6 MB/core at fp16): x and all expert weights
are cast to fp16 on the host (11-bit mantissa keeps final rel-err ~3e-4)
and packed so every load is a contiguous DMA. The router path stays fp32
end-to-end for exact argmax selection.

Layout: features on partitions, batch on the free dim for l1/l2. The l3
and gate matmuls are emitted "transposed" (activation/router tiles as
the stationary operand, tiny [*, 8] weights moving) so outputs land
directly in batch-on-partitions layout and no PE transposes are needed.

Scheduling notes (from trace analysis):
- dma_start costs ~0.6-1.5us of issue time on the issuing engine, so
  loads are consolidated into a few large DMAs split across the two
  HWDGE queues (sync: x pieces; scalar: weights/router, issued before
  any ACT compute op so the ACT FIFO can't head-of-line-block them).
- The PE HAM clock gate defaults to half rate and only warms after
  ~3.4us of sustained matmul activity; a warmup burst of dummy matmuls
  during the initial DMA window gets the PE to 2.4 GHz before real work.
- tail(b-1) PE work (l2/l3/gate matmuls) is interleaved into burst(b)'s
  chunk stream so tails complete during the next block's DMA window;
  only the last block's tail chain runs after the final x bytes land.
"""

import os
from contextlib import ExitStack

import numpy as np

import concourse.bacc as bacc
import concourse.mybir as mybir
import concourse.tile as tile

N_CORES = 8
B, L1, L2, L3, E = 16384, 3072, 15, 32, 8
RF = 32  # router feats per perspective
HALF = L1 // 2
B_SH = B // N_CORES  # 2048 rows per core
KC = L1 // 128  # 24 contraction chunks
SQ_SCALE = float(np.sqrt(255.0 / 256.0))
MB = 512
NB = B_SH // MB  # 4 blocks
NSUB = MB // 128  # 4 sub-blocks per block

# fp16 const pack layout (columns of cw16)
CW_W1 = 0  # [128, KC*128] w1t
CW_W2 = KC * 128  # [128, 512] l2 weights
CW_W3 = CW_W2 + 512  # [128, 16] l3 weights
CW_ID = CW_W3 + 16  # [128, 8] identity (rows 0:8)
CW_WR = CW_ID + 8  # [65, 8] fp16 router weights + bias row
CW_END = CW_WR + 8
# fp32 const pack layout (columns of c32)
C32_WR = 0  # [65, 8] router weights + bias row
C32_BIAS = 8  # [128, 5] bias columns
C32_END = 16

# x piece structure per block: list of chunk counts. Small (~786 KB)
# pieces matter: the HWDGE ring holds ~4 in-flight DMAs that progress
# concurrently (packet round-robin), so a piece's completion semaphore
# fires roughly (in-flight bytes)/BW after its issue — big pieces starve
# consumers. The trailing 3-chunk pieces shrink the final tail's gate.
PIECES = {0: [3, 3, 6, 6, 6], 1: [6, 6, 6, 6], 2: [6, 6, 6, 6], 3: [6, 6, 6, 3, 2, 1]}

F32 = mybir.dt.float32
F16 = mybir.dt.float16
ALU = mybir.AluOpType
AF = mybir.ActivationFunctionType

N_WARM = 40


def build_nc():
    nc = bacc.Bacc(dynamic_dma_scratch_size=2048)

    xq = nc.dram_tensor("xq", [128, NB * KC * MB], F16, kind="ExternalInput")
    xr = nc.dram_tensor("xr", [RF * 2 + 1, NB * MB], F16, kind="ExternalInput")
    cw16 = nc.dram_tensor("cw16", [128, CW_END], F16, kind="ExternalInput")
    c32 = nc.dram_tensor("c32", [128, C32_END], F32, kind="ExternalInput")
    y = nc.dram_tensor("y", [128, NB * NSUB], F32, kind="ExternalOutput")

    with tile.TileContext(nc) as tc, ExitStack() as ctx:
        const = ctx.enter_context(tc.tile_pool(name="const", bufs=1))
        xp6 = ctx.enter_context(tc.tile_pool(name="x6", bufs=15))
        xp3 = ctx.enter_context(tc.tile_pool(name="x3", bufs=4))
        xp2 = ctx.enter_context(tc.tile_pool(name="x2", bufs=1))
        xp1 = ctx.enter_context(tc.tile_pool(name="x1", bufs=1))
        actp = ctx.enter_context(tc.tile_pool(name="act", bufs=2))
        selp = ctx.enter_context(tc.tile_pool(name="sel", bufs=2))
        ps_w = ctx.enter_context(tc.tile_pool(name="psw", bufs=1, space="PSUM"))
        ps_1 = ctx.enter_context(tc.tile_pool(name="ps1", bufs=2, space="PSUM"))
        ps_2a = ctx.enter_context(tc.tile_pool(name="ps2a", bufs=1, space="PSUM"))
        ps_2b = ctx.enter_context(tc.tile_pool(name="ps2b", bufs=1, space="PSUM"))
        ps_g = ctx.enter_context(tc.tile_pool(name="psg", bufs=1, space="PSUM"))
        ps_sel = ctx.enter_context(tc.tile_pool(name="pssel", bufs=2, space="PSUM"))

        cw = const.tile([128, CW_END], F16)
        c3 = const.tile([128, C32_END], F32)
        xr_sb = const.tile([RF * 2 + 1, NB * MB], F16)
        scratch = const.tile([128, 256], F16)

        def w1(c):  # lhsT for l1 chunk c
            return cw[:, CW_W1 + c * 128 : CW_W1 + (c + 1) * 128]

        # --- DMAs, all issued up front ---
        # scalar consts FIRST: DMA-completion semaphore lanes (8) are
        # assigned round-robin in emission order, and a lane's next DMA
        # issue waits on its predecessor's completion — consts emitted
        # after the x pieces would block on mid-kernel x completions
        # (head-of-line blocking the whole ACT queue with them).
        nc.scalar.dma_start(cw[:, 0:1536], cw16[:, 0:1536])
        nc.scalar.dma_start(cw[:, 1536:CW_END], cw16[:, 1536:CW_END])
        nc.scalar.dma_start(c3[:], c32[:, :])

        # sync: x pieces
        xt = {}  # (b, piece_idx) -> (tile, c0, n)

        def load_x(b, pi, c0, n, pool):
            t = pool.tile([128, n, MB], F16, tag=f"xt{n}")
            g0 = b * KC + c0
            nc.sync.dma_start(
                t[:],
                xq[:, g0 * MB : (g0 + n) * MB].rearrange("p (c m) -> p c m", m=MB),
            )
            xt[(b, pi)] = (t, c0, n)

        pools = {6: xp6, 3: xp3, 2: xp2, 1: xp1}
        for b in range(NB):
            c0 = 0
            for pi, n in enumerate(PIECES[b]):
                load_x(b, pi, c0, n, pools[n])
                c0 += n
            if b == 0:
                nc.sync.dma_start(xr_sb[:], xr[:, :])

        # --- PE warmup: dummy matmuls to release the HAM clock gate ---
        nc.vector.memset(scratch[:], 0.0)
        warm_ps = ps_w.tile([64, 128], F32)
        for _ in range(N_WARM):
            nc.tensor.matmul(
                warm_ps[:], scratch[:, 0:64], scratch[:, 0:128],
                start=True, stop=True,
            )

        st = {}

        def chunk_rhs(b, c):
            for t, c0, n in (xt[(b, p)] for p in range(len(PIECES[b]))):
                if c0 <= c < c0 + n:
                    return t[:, c - c0, :]
            raise KeyError((b, c))

        def emit_burst(b, clo, chi):
            if clo == 0:
                ps1_t = ps_1.tile([128, MB], F32, tag="ps1")
                st[b] = {"ps1": ps1_t}
            ps1 = st[b]["ps1"]
            for c in range(clo, chi):
                nc.tensor.matmul(
                    ps1[:], w1(c), chunk_rhs(b, c),
                    start=(c == 0), stop=(c == KC - 1),
                )

        def emit_tail_front(b):
            # engine split: sq + lout on ACT while raw runs on DVE
            ps1 = st[b]["ps1"]
            sq = actp.tile([128, MB], F16, tag="sq")
            nc.scalar.activation(
                sq[:], ps1[:], AF.Square, bias=c3[:, C32_BIAS : C32_BIAS + 1], scale=SQ_SCALE
            )
            lout = actp.tile([E, MB], F16, tag="lout")
            nc.scalar.activation(
                lout[:], ps1[0:E, :], AF.Identity, bias=c3[0:E, C32_BIAS + 4 : C32_BIAS + 5]
            )
            raw = actp.tile([128, MB], F16, tag="raw")
            nc.vector.tensor_scalar(
                raw[:], ps1[:], c3[:, C32_BIAS + 1 : C32_BIAS + 2], 0.0,
                op0=ALU.add, op1=ALU.max,
            )
            nc.vector.tensor_scalar_min(raw[:], raw[:], 1.0)
            nc.vector.tensor_scalar_min(sq[:], sq[:], 1.0)
            st[b].update(raw=raw, sq=sq, lout=lout)

        def emit_tail_l2(b):
            raw, sq = st[b]["raw"], st[b]["sq"]
            ps2a = ps_2a.tile([128, MB], F32, tag="ps2a")
            nc.tensor.matmul(ps2a[:], cw[:, CW_W2 + 128 : CW_W2 + 256], raw[:],
                             start=True, stop=False)
            nc.tensor.matmul(ps2a[:], cw[:, CW_W2 : CW_W2 + 128], sq[:],
                             start=False, stop=True)
            ps2b = ps_2b.tile([128, MB], F32, tag="ps2b")
            nc.tensor.matmul(ps2b[:], cw[:, CW_W2 + 384 : CW_W2 + 512], raw[:],
                             start=True, stop=False)
            nc.tensor.matmul(ps2b[:], cw[:, CW_W2 + 256 : CW_W2 + 384], sq[:],
                             start=False, stop=True)
            l2a = actp.tile([128, MB], F16, tag="l2a")
            nc.scalar.activation(l2a[:], ps2a[:], AF.Relu, bias=c3[:, C32_BIAS + 2 : C32_BIAS + 3])
            nc.vector.tensor_scalar_min(l2a[:], l2a[:], 1.0)
            l2b = actp.tile([128, MB], F16, tag="l2b")
            nc.vector.tensor_scalar(
                l2b[:], ps2b[:], c3[:, C32_BIAS + 3 : C32_BIAS + 4], 0.0,
                op0=ALU.add, op1=ALU.max,
            )
            nc.vector.tensor_scalar_min(l2b[:], l2b[:], 1.0)
            st[b].update(l2a=l2a, l2b=l2b)

        def emit_gates(b):
            # gate matmuls for block b, interleaved into block-0's piece
            # gaps (they only need xr, loaded early) so they soak up PE
            # idle during the DMA ramp instead of delaying l1 work.
            if b == 0:
                gp = ps_g.tile([128, NB, NSUB, 32], F32)
                st["gps"] = gp
            gp = st["gps"]
            for j in range(NSUB):
                nc.tensor.matmul(
                    gp[:, b, j, 0:E],
                    xr_sb[0:65, b * MB + j * 128 : b * MB + (j + 1) * 128],
                    cw[0:65, CW_WR : CW_WR + 8],
                    start=True, stop=True,
                )

        def emit_select_pre():
            # mxv reads the whole gate bank, so the DVE ops are FIFO-ordered
            # after every PE write to that bank (no PE-W/DVE-R bank race).
            gp = st["gps"]
            mxv = const.tile([128, NB * NSUB], F32)
            nc.vector.reduce_max(
                mxv[:], gp[:, :, :, 0:E], axis=mybir.AxisListType.X
            )
            eqall = const.tile([128, NB, NSUB, E], F32)
            for b in range(NB):
                for j in range(NSUB):
                    k = b * NSUB + j
                    nc.vector.tensor_scalar(
                        eqall[:, b, j, :], gp[:, b, j, 0:E],
                        mxv[:, k : k + 1], None, op0=ALU.is_ge,
                    )
            st["eq"] = eqall

        def emit_tail_l3(b):
            l2a, l2b, lout = st[b]["l2a"], st[b]["l2b"], st[b]["lout"]
            comb = ps_sel.tile([128, NSUB, 128], F32, tag="comb")
            for j in range(NSUB):
                js = slice(j * 128, (j + 1) * 128)
                nc.tensor.matmul(comb[:, j, 0:E], l2a[:, js],
                                 cw[:, CW_W3 : CW_W3 + 8], start=True, stop=False)
                nc.tensor.matmul(comb[:, j, 0:E], l2b[:, js],
                                 cw[:, CW_W3 + 8 : CW_W3 + 16], start=False,
                                 stop=False)
                nc.tensor.matmul(comb[:, j, 0:E], lout[:, js],
                                 cw[0:E, CW_ID : CW_ID + 8], start=False, stop=True)
            prod = selp.tile([128, NSUB, E], F32, tag="prod")
            nc.vector.tensor_tensor(
                prod[:], st["eq"][:, b], comb[:, :, 0:E], op=ALU.mult
            )
            yb = selp.tile([128, NSUB], F32, tag="yb")
            nc.vector.reduce_sum(yb[:], prod[:], axis=mybir.AxisListType.X)
            nc.sync.dma_start(y[:, b * NSUB : (b + 1) * NSUB], yb[:])
            del st[b]

        # --- pipeline ---
        emit_burst(0, 0, 6)
        emit_gates(0)
        emit_burst(0, 6, 12)
        emit_gates(1)
        emit_burst(0, 12, 18)
        emit_gates(2)
        emit_burst(0, 18, KC)
        emit_gates(3)
        emit_tail_front(0)
        emit_select_pre()
        for b in range(1, NB):
            if b > 1:
                emit_tail_front(b - 1)
            emit_burst(b, 0, 9)
            emit_tail_l2(b - 1)
            emit_burst(b, 9, 15)
            emit_tail_l3(b - 1)
            emit_burst(b, 15, KC)
        emit_tail_front(NB - 1)
        emit_tail_l2(NB - 1)
        emit_tail_l3(NB - 1)

    nc.finalize()
    return nc


def prep_weights(router_w, router_b, l1_w, l1_b, l2_w, l2_b, out_w, out_b):
    """Host-side packing of the (tiny) weights into the kernel's layouts."""
    f4, f2 = np.float32, np.float16
    # W1 stacked: row f = e for o=15 (l1x_out), f = 8 + o*8 + e for o < 15
    w1_stacked = np.concatenate(
        [l1_w[:, L2, :], np.transpose(l1_w[:, :L2, :], (1, 0, 2)).reshape(120, L1)],
        axis=0,
    )  # [128, L1]
    w1t_kf = np.ascontiguousarray(w1_stacked.T)  # [L1, 128]
    w1t = np.ascontiguousarray(
        np.transpose(w1t_kf.reshape(KC, 128, 128), (1, 0, 2))
    ).reshape(128, KC * 128)
    # l2 block weights: rows f_in = 8+o*8+e, packed [sqA | rawA | sqB | rawB]
    w2p = np.zeros((128, 512), f4)
    for e in range(E):
        base = 0 if e < 4 else 256
        c0 = (e % 4) * 32
        wt = l2_w[e].T  # [30, 32]; rows 0..14 sq features, 15..29 raw
        rows = 8 + np.arange(L2) * 8 + e  # f for o in 0..14
        w2p[rows, base + c0 : base + c0 + 32] = wt[0:L2]
        w2p[rows, base + 128 + c0 : base + 128 + c0 + 32] = wt[L2 : 2 * L2]
    # l3: [128, 16] = [W3A | W3B]; transposed-l3 contracts over the 128
    # l2-feature rows, out col = expert.
    w3p = np.zeros((128, 16), f4)
    for e in range(E):
        col = e if e < 4 else 8 + e
        w3p[(e % 4) * 32 : (e % 4) * 32 + 32, col] = out_w[e, 0, :]
    cw16 = np.zeros((128, CW_END), f2)
    cw16[:, CW_W1 : CW_W1 + KC * 128] = w1t
    cw16[:, CW_W2 : CW_W2 + 512] = w2p
    cw16[:, CW_W3 : CW_W3 + 16] = w3p
    cw16[0:E, CW_ID : CW_ID + 8] = np.eye(E)
    cw16[0 : 2 * RF, CW_WR : CW_WR + 8] = router_w.T
    cw16[2 * RF, CW_WR : CW_WR + 8] = router_b
    # fp32 consts: router (with trailing ones-row bias) + ACT bias columns
    c32 = np.zeros((128, C32_END), f4)
    c32[0 : 2 * RF, C32_WR : C32_WR + 8] = router_w.T
    c32[2 * RF, C32_WR : C32_WR + 8] = router_b
    b1col = np.concatenate([l1_b[:, L2], l1_b[:, :L2].T.reshape(120)])
    c32[:, C32_BIAS + 0] = SQ_SCALE * b1col  # Square bias: (s*x + s*b1)^2
    c32[:, C32_BIAS + 1] = b1col  # raw Relu bias
    c32[:, C32_BIAS + 2] = l2_b[0:4].reshape(128)
    c32[:, C32_BIAS + 3] = l2_b[4:8].reshape(128)
    c32[0:E, C32_BIAS + 4] = l1_b[:, L2] + out_b[:, 0]  # lout bias
    return {"cw16": cw16, "c32": c32}


_cache = {}
_last_results = None


def kernel(x, router_w, router_b, l1_w, l1_b, l2_w, l2_b, out_w, out_b):
    global _last_results
    x = np.asarray(x, dtype=np.float32)
    weights = prep_weights(
        np.asarray(router_w, np.float32),
        np.asarray(router_b, np.float32),
        np.asarray(l1_w, np.float32),
        np.asarray(l1_b, np.float32),
        np.asarray(l2_w, np.float32),
        np.asarray(l2_b, np.float32),
        np.asarray(out_w, np.float32),
        np.asarray(out_b, np.float32),
    )

    # router input (fp16): first 32 feats of each perspective + ones row
    xr_full = np.empty((2 * RF + 1, B), np.float16)
    xr_full[0:RF] = x[:, 0:RF].T
    xr_full[RF : 2 * RF] = x[:, HALF : HALF + RF].T
    xr_full[2 * RF] = 1.0

    in_maps = []
    for c in range(N_CORES):
        shard = x[c * B_SH : (c + 1) * B_SH]  # [2048, 3072]
        xh = shard.astype(np.float16)
        # xq[p, b, c, m] = shard[b*MB + m, c*128 + p]
        xqc = np.ascontiguousarray(
            xh.T.reshape(KC, 128, NB, MB).transpose(1, 2, 0, 3)
        ).reshape(128, NB * KC * MB)
        xrc = np.ascontiguousarray(xr_full[:, c * B_SH : (c + 1) * B_SH])
        in_maps.append({"xq": xqc, "xr": xrc, **weights})

    if "nc" not in _cache:
        _cache["nc"] = build_nc()
    nc = _cache["nc"]

    from concourse.bass_utils import run_bass_kernel_spmd

    trace = bool(int(os.environ.get("KERNEL_TRACE", "0")))
    try:
        res = run_bass_kernel_spmd(
            nc, in_maps, core_ids=list(range(N_CORES)), trace=trace
        )
    except Exception:
        if not trace:
            raise
        res = run_bass_kernel_spmd(
            nc, in_maps, core_ids=list(range(N_CORES)), trace=False
        )
    _last_results = res
    out = np.concatenate(
        [np.ascontiguousarray(r["y"].T).reshape(B_SH, 1) for r in res.results], axis=0
    )
    return out


# revision 24
# speedup vs baseline: 1.0974x; 1.0974x over previous
"""Trainium2 Bass kernel for nn_MoELayerStacks (moe_routing).

Full inputs in, full output out. Data-parallel over batch across 8 cores.

Math (per batch row b):
  gate = [x[:32], x[1536:1568]] @ router_w.T + router_b           # [8]
  idx  = argmax(gate)
  l1c  = x @ l1_w[e].T + l1_b[e]   for all e                      # [8, 16]
  l1x  = clip([square(l1c[:, :15])*255/256, l1c[:, :15]], 0, 1)   # [8, 30]
  l2x  = clip(l1x @ l2_w[e].T + l2_b[e], 0, 1)                    # [8, 32]
  out  = (l2x @ out_w[e].T + out_b[e] + l1c[:, 15])[idx]          # [1]

HBM-bound on reading x (12.6 MB/core at fp16): x and all expert weights
are cast to fp16 on the host (11-bit mantissa keeps final rel-err ~3e-4)
and packed so every load is a contiguous DMA. The router path stays fp32
end-to-end for exact argmax selection.

Layout: features on partitions, batch on the free dim for l1/l2. The l3
and gate matmuls are emitted "transposed" (activation/router tiles as
the stationary operand, tiny [*, 8] weights moving) so outputs land
directly in batch-on-partitions layout and no PE transposes are needed.

Scheduling notes (from trace analysis):
- dma_start costs ~0.6-1.5us of issue time on the issuing engine, so
  loads are consolidated into a few large DMAs split across the two
  HWDGE queues (sync: x pieces; scalar: weights/router, issued before
  any ACT compute op so the ACT FIFO can't head-of-line-block them).
- The PE HAM clock gate defaults to half rate and only warms after
  ~3.4us of sustained matmul activity; a warmup burst of dummy matmuls
  during the initial DMA window gets the PE to 2.4 GHz before real work.
- tail(b-1) PE work (l2/l3/gate matmuls) is interleaved into burst(b)'s
  chunk stream so tails complete during the next block's DMA window;
  only the last block's tail chain runs after the final x bytes land.
"""

import os
from contextlib import ExitStack

import numpy as np

import concourse.bacc as bacc
import concourse.mybir as mybir
import concourse.tile as tile

N_CORES = 8
B, L1, L2, L3, E = 16384, 3072, 15, 32, 8
RF = 32  # router feats per perspective
HALF = L1 // 2
B_SH = B // N_CORES  # 2048 rows per core
KC = L1 // 128  # 24 contraction chunks
SQ_SCALE = float(np.sqrt(255.0 / 256.0))
MB = 512
NB = B_SH // MB  # 4 blocks
NSUB = MB // 128  # 4 sub-blocks per block

# fp16 const pack layout (columns of cw16)
CW_W1 = 0  # [128, KC*128] w1t
CW_W2 = KC * 128  # [128, 512] l2 weights
CW_W3 = CW_W2 + 512  # [128, 16] l3 weights
CW_ID = CW_W3 + 16  # [128, 8] identity (rows 0:8)
CW_WR = CW_ID + 8  # [65, 8] fp16 router weights + bias row
CW_END = CW_WR + 8
# fp32 const pack layout (columns of c32)
C32_WR = 0  # [65, 8] router weights + bias row
C32_BIAS = 8  # [128, 5] bias columns
C32_END = 16

# x piece structure per block: list of chunk counts. Small (~786 KB)
# pieces matter: the HWDGE ring holds ~4 in-flight DMAs that progress
# concurrently (packet round-robin), so a piece's completion semaphore
# fires roughly (in-flight bytes)/BW after its issue — big pieces starve
# consumers. The trailing 3-chunk pieces shrink the final tail's gate.
PIECES = {0: [3, 3, 6, 6, 6], 1: [6, 6, 6, 6], 2: [6, 6, 6, 6], 3: [6, 6, 6, 3, 3]}

F32 = mybir.dt.float32
F16 = mybir.dt.float16
ALU = mybir.AluOpType
AF = mybir.ActivationFunctionType

N_WARM = 40


def build_nc():
    nc = bacc.Bacc(dynamic_dma_scratch_size=2048)

    xq = nc.dram_tensor("xq", [128, NB * KC * MB], F16, kind="ExternalInput")
    xr = nc.dram_tensor("xr", [RF * 2 + 1, NB * MB], F16, kind="ExternalInput")
    cw16 = nc.dram_tensor("cw16", [128, CW_END], F16, kind="ExternalInput")
    c32 = nc.dram_tensor("c32", [128, C32_END], F32, kind="ExternalInput")
    y = nc.dram_tensor("y", [128, NB * NSUB], F32, kind="ExternalOutput")

    with tile.TileContext(nc) as tc, ExitStack() as ctx:
        const = ctx.enter_context(tc.tile_pool(name="const", bufs=1))
        xp6 = ctx.enter_context(tc.tile_pool(name="x6", bufs=15))
        xp3 = ctx.enter_context(tc.tile_pool(name="x3", bufs=4))
        xp2 = ctx.enter_context(tc.tile_pool(name="x2", bufs=1))
        xp1 = ctx.enter_context(tc.tile_pool(name="x1", bufs=1))
        actp = ctx.enter_context(tc.tile_pool(name="act", bufs=2))
        selp = ctx.enter_context(tc.tile_pool(name="sel", bufs=2))
        ps_w = ctx.enter_context(tc.tile_pool(name="psw", bufs=1, space="PSUM"))
        ps_1 = ctx.enter_context(tc.tile_pool(name="ps1", bufs=2, space="PSUM"))
        ps_2a = ctx.enter_context(tc.tile_pool(name="ps2a", bufs=1, space="PSUM"))
        ps_2b = ctx.enter_context(tc.tile_pool(name="ps2b", bufs=1, space="PSUM"))
        ps_g = ctx.enter_context(tc.tile_pool(name="psg", bufs=1, space="PSUM"))
        ps_sel = ctx.enter_context(tc.tile_pool(name="pssel", bufs=2, space="PSUM"))

        cw = const.tile([128, CW_END], F16)
        c3 = const.tile([128, C32_END], F32)
        xr_sb = const.tile([RF * 2 + 1, NB * MB], F16)
        scratch = const.tile([128, 256], F16)

        def w1(c):  # lhsT for l1 chunk c
            return cw[:, CW_W1 + c * 128 : CW_W1 + (c + 1) * 128]

        # --- DMAs, all issued up front ---
        # scalar consts FIRST: DMA-completion semaphore lanes (8) are
        # assigned round-robin in emission order, and a lane's next DMA
        # issue waits on its predecessor's completion — consts emitted
        # after the x pieces would block on mid-kernel x completions
        # (head-of-line blocking the whole ACT queue with them).
        nc.scalar.dma_start(cw[:, 0:1536], cw16[:, 0:1536])
        nc.scalar.dma_start(cw[:, 1536:CW_END], cw16[:, 1536:CW_END])
        nc.scalar.dma_start(c3[:], c32[:, :])

        # sync: x pieces
        xt = {}  # (b, piece_idx) -> (tile, c0, n)

        def load_x(b, pi, c0, n, pool):
            t = pool.tile([128, n, MB], F16, tag=f"xt{n}")
            g0 = b * KC + c0
            nc.sync.dma_start(
                t[:],
                xq[:, g0 * MB : (g0 + n) * MB].rearrange("p (c m) -> p c m", m=MB),
            )
            xt[(b, pi)] = (t, c0, n)

        pools = {6: xp6, 3: xp3, 2: xp2, 1: xp1}
        for b in range(NB):
            c0 = 0
            for pi, n in enumerate(PIECES[b]):
                load_x(b, pi, c0, n, pools[n])
                c0 += n
                if b == 0 and pi == 1:
                    nc.sync.dma_start(xr_sb[:], xr[:, :])


        # --- PE warmup: dummy matmuls to release the HAM clock gate ---
        nc.vector.memset(scratch[:], 0.0)
        warm_ps = ps_w.tile([64, 128], F32)
        for _ in range(N_WARM):
            nc.tensor.matmul(
                warm_ps[:], scratch[:, 0:64], scratch[:, 0:128],
                start=True, stop=True,
            )

        st = {}

        def chunk_rhs(b, c):
            for t, c0, n in (xt[(b, p)] for p in range(len(PIECES[b]))):
                if c0 <= c < c0 + n:
                    return t[:, c - c0, :]
            raise KeyError((b, c))

        def emit_burst(b, clo, chi):
            if clo == 0:
                ps1_t = ps_1.tile([128, MB], F32, tag="ps1")
                st[b] = {"ps1": ps1_t}
            ps1 = st[b]["ps1"]
            for c in range(clo, chi):
                nc.tensor.matmul(
                    ps1[:], w1(c), chunk_rhs(b, c),
                    start=(c == 0), stop=(c == KC - 1),
                )

        def emit_tail_front(b):
            # ACT (raw first so the first l2 matmul can start earliest)
            ps1 = st[b]["ps1"]
            raw = actp.tile([128, MB], F16, tag="raw")
            nc.scalar.activation(raw[:], ps1[:], AF.Relu, bias=c3[:, C32_BIAS + 1 : C32_BIAS + 2])
            sq = actp.tile([128, MB], F16, tag="sq")
            nc.scalar.activation(
                sq[:], ps1[:], AF.Square, bias=c3[:, C32_BIAS : C32_BIAS + 1], scale=SQ_SCALE
            )
            lout = actp.tile([E, MB], F16, tag="lout")
            nc.scalar.activation(
                lout[:], ps1[0:E, :], AF.Identity, bias=c3[0:E, C32_BIAS + 4 : C32_BIAS + 5]
            )
            nc.vector.tensor_scalar_min(raw[:], raw[:], 1.0)
            nc.vector.tensor_scalar_min(sq[:], sq[:], 1.0)
            st[b].update(raw=raw, sq=sq, lout=lout)

        def emit_tail_l2(b):
            raw, sq = st[b]["raw"], st[b]["sq"]
            ps2a = ps_2a.tile([128, MB], F32, tag="ps2a")
            nc.tensor.matmul(ps2a[:], cw[:, CW_W2 + 128 : CW_W2 + 256], raw[:],
                             start=True, stop=False)
            nc.tensor.matmul(ps2a[:], cw[:, CW_W2 : CW_W2 + 128], sq[:],
                             start=False, stop=True)
            ps2b = ps_2b.tile([128, MB], F32, tag="ps2b")
            nc.tensor.matmul(ps2b[:], cw[:, CW_W2 + 384 : CW_W2 + 512], raw[:],
                             start=True, stop=False)
            nc.tensor.matmul(ps2b[:], cw[:, CW_W2 + 256 : CW_W2 + 384], sq[:],
                             start=False, stop=True)
            l2a = actp.tile([128, MB], F16, tag="l2a")
            nc.scalar.activation(l2a[:], ps2a[:], AF.Relu, bias=c3[:, C32_BIAS + 2 : C32_BIAS + 3])
            nc.vector.tensor_scalar_min(l2a[:], l2a[:], 1.0)
            l2b = actp.tile([128, MB], F16, tag="l2b")
            nc.scalar.activation(l2b[:], ps2b[:], AF.Relu, bias=c3[:, C32_BIAS + 3 : C32_BIAS + 4])
            nc.vector.tensor_scalar_min(l2b[:], l2b[:], 1.0)
            st[b].update(l2a=l2a, l2b=l2b)

        def emit_gates(b):
            # gate matmuls for block b, interleaved into block-0's piece
            # gaps (they only need xr, loaded early) so they soak up PE
            # idle during the DMA ramp instead of delaying l1 work.
            if b == 0:
                gp = ps_g.tile([128, NB, NSUB, 32], F32)
                st["gps"] = gp
            gp = st["gps"]
            for j in range(NSUB):
                nc.tensor.matmul(
                    gp[:, b, j, 0:E],
                    xr_sb[0:65, b * MB + j * 128 : b * MB + (j + 1) * 128],
                    cw[0:65, CW_WR : CW_WR + 8],
                    start=True, stop=True,
                )

        def emit_select_pre():
            # mxv reads the whole gate bank, so the DVE ops are FIFO-ordered
            # after every PE write to that bank (no PE-W/DVE-R bank race).
            gp = st["gps"]
            mxv = const.tile([128, NB * NSUB], F32)
            nc.vector.reduce_max(
                mxv[:], gp[:, :, :, 0:E], axis=mybir.AxisListType.X
            )
            eqall = const.tile([128, NB, NSUB, E], F32)
            for b in range(NB):
                for j in range(NSUB):
                    k = b * NSUB + j
                    nc.vector.tensor_scalar(
                        eqall[:, b, j, :], gp[:, b, j, 0:E],
                        mxv[:, k : k + 1], None, op0=ALU.is_ge,
                    )
            st["eq"] = eqall

        def emit_tail_l3(b):
            l2a, l2b, lout = st[b]["l2a"], st[b]["l2b"], st[b]["lout"]
            comb = ps_sel.tile([128, NSUB, 128], F32, tag="comb")
            for j in range(NSUB):
                js = slice(j * 128, (j + 1) * 128)
                nc.tensor.matmul(comb[:, j, 0:E], l2a[:, js],
                                 cw[:, CW_W3 : CW_W3 + 8], start=True, stop=False)
                nc.tensor.matmul(comb[:, j, 0:E], l2b[:, js],
                                 cw[:, CW_W3 + 8 : CW_W3 + 16], start=False,
                                 stop=False)
                nc.tensor.matmul(comb[:, j, 0:E], lout[:, js],
                                 cw[0:E, CW_ID : CW_ID + 8], start=False, stop=True)
            prod = selp.tile([128, NSUB, E], F32, tag="prod")
            nc.vector.tensor_tensor(
                prod[:], st["eq"][:, b], comb[:, :, 0:E], op=ALU.mult
            )
            yb = selp.tile([128, NSUB], F32, tag="yb")
            nc.vector.reduce_sum(yb[:], prod[:], axis=mybir.AxisListType.X)
            nc.sync.dma_start(y[:, b * NSUB : (b + 1) * NSUB], yb[:])
            del st[b]

        # --- pipeline ---
        emit_burst(0, 0, 6)
        emit_gates(0)
        emit_burst(0, 6, 12)
        emit_gates(1)
        emit_burst(0, 12, 18)
        emit_gates(2)
        emit_burst(0, 18, KC)
        emit_gates(3)
        emit_tail_front(0)
        emit_select_pre()
        for b in range(1, NB):
            if b > 1:
                emit_tail_front(b - 1)
            emit_burst(b, 0, 9)
            emit_tail_l2(b - 1)
            emit_burst(b, 9, 15)
            emit_tail_l3(b - 1)
            emit_burst(b, 15, KC)
        emit_tail_front(NB - 1)
        emit_tail_l2(NB - 1)
        emit_tail_l3(NB - 1)

    nc.finalize()
    return nc


def prep_weights(router_w, router_b, l1_w, l1_b, l2_w, l2_b, out_w, out_b):
    """Host-side packing of the (tiny) weights into the kernel's layouts."""
    f4, f2 = np.float32, np.float16
    # W1 stacked: row f = e for o=15 (l1x_out), f = 8 + o*8 + e for o < 15
    w1_stacked = np.concatenate(
        [l1_w[:, L2, :], np.transpose(l1_w[:, :L2, :], (1, 0, 2)).reshape(120, L1)],
        axis=0,
    )  # [128, L1]
    w1t_kf = np.ascontiguousarray(w1_stacked.T)  # [L1, 128]
    w1t = np.ascontiguousarray(
        np.transpose(w1t_kf.reshape(KC, 128, 128), (1, 0, 2))
    ).reshape(128, KC * 128)
    # l2 block weights: rows f_in = 8+o*8+e, packed [sqA | rawA | sqB | rawB]
    w2p = np.zeros((128, 512), f4)
    for e in range(E):
        base = 0 if e < 4 else 256
        c0 = (e % 4) * 32
        wt = l2_w[e].T  # [30, 32]; rows 0..14 sq features, 15..29 raw
        rows = 8 + np.arange(L2) * 8 + e  # f for o in 0..14
        w2p[rows, base + c0 : base + c0 + 32] = wt[0:L2]
        w2p[rows, base + 128 + c0 : base + 128 + c0 + 32] = wt[L2 : 2 * L2]
    # l3: [128, 16] = [W3A | W3B]; transposed-l3 contracts over the 128
    # l2-feature rows, out col = expert.
    w3p = np.zeros((128, 16), f4)
    for e in range(E):
        col = e if e < 4 else 8 + e
        w3p[(e % 4) * 32 : (e % 4) * 32 + 32, col] = out_w[e, 0, :]
    cw16 = np.zeros((128, CW_END), f2)
    cw16[:, CW_W1 : CW_W1 + KC * 128] = w1t
    cw16[:, CW_W2 : CW_W2 + 512] = w2p
    cw16[:, CW_W3 : CW_W3 + 16] = w3p
    cw16[0:E, CW_ID : CW_ID + 8] = np.eye(E)
    cw16[0 : 2 * RF, CW_WR : CW_WR + 8] = router_w.T
    cw16[2 * RF, CW_WR : CW_WR + 8] = router_b
    # fp32 consts: router (with trailing ones-row bias) + ACT bias columns
    c32 = np.zeros((128, C32_END), f4)
    c32[0 : 2 * RF, C32_WR : C32_WR + 8] = router_w.T
    c32[2 * RF, C32_WR : C32_WR + 8] = router_b
    b1col = np.concatenate([l1_b[:, L2], l1_b[:, :L2].T.reshape(120)])
    c32[:, C32_BIAS + 0] = SQ_SCALE * b1col  # Square bias: (s*x + s*b1)^2
    c32[:, C32_BIAS + 1] = b1col  # raw Relu bias
    c32[:, C32_BIAS + 2] = l2_b[0:4].reshape(128)
    c32[:, C32_BIAS + 3] = l2_b[4:8].reshape(128)
    c32[0:E, C32_BIAS + 4] = l1_b[:, L2] + out_b[:, 0]  # lout bias
    return {"cw16": cw16, "c32": c32}


_cache = {}
_last_results = None


def kernel(x, router_w, router_b, l1_w, l1_b, l2_w, l2_b, out_w, out_b):
    global _last_results
    x = np.asarray(x, dtype=np.float32)
    weights = prep_weights(
        np.asarray(router_w, np.float32),
        np.asarray(router_b, np.float32),
        np.asarray(l1_w, np.float32),
        np.asarray(l1_b, np.float32),
        np.asarray(l2_w, np.float32),
        np.asarray(l2_b, np.float32),
        np.asarray(out_w, np.float32),
        np.asarray(out_b, np.float32),
    )

    # router input (fp16): first 32 feats of each perspective + ones row
    xr_full = np.empty((2 * RF + 1, B), np.float16)
    xr_full[0:RF] = x[:, 0:RF].T
    xr_full[RF : 2 * RF] = x[:, HALF : HALF + RF].T
    xr_full[2 * RF] = 1.0

    in_maps = []
    for c in range(N_CORES):
        shard = x[c * B_SH : (c + 1) * B_SH]  # [2048, 3072]
        xh = shard.astype(np.float16)
        # xq[p, b, c, m] = shard[b*MB + m, c*128 + p]
        xqc = np.ascontiguousarray(
            xh.T.reshape(KC, 128, NB, MB).transpose(1, 2, 0, 3)
        ).reshape(128, NB * KC * MB)
        xrc = np.ascontiguousarray(xr_full[:, c * B_SH : (c + 1) * B_SH])
        in_maps.append({"xq": xqc, "xr": xrc, **weights})

    if "nc" not in _cache:
        _cache["nc"] = build_nc()
    nc = _cache["nc"]

    from concourse.bass_utils import run_bass_kernel_spmd

    trace = bool(int(os.environ.get("KERNEL_TRACE", "0")))
    try:
        res = run_bass_kernel_spmd(
            nc, in_maps, core_ids=list(range(N_CORES)), trace=trace
        )
    except Exception:
        if not trace:
            raise
        res = run_bass_kernel_spmd(
            nc, in_maps, core_ids=list(range(N_CORES)), trace=False
        )
    _last_results = res
    out = np.concatenate(
        [np.ascontiguousarray(r["y"].T).reshape(B_SH, 1) for r in res.results], axis=0
    )
    return out
